# revision 1
# baseline (speedup 1.0000x reference)
"""Trainium2 Bass kernel for nn_CategoryHead (tiny 4-layer post-norm
transformer classifier head over B=65536 samples, T=2 tokens, D=128).

Strategy: pure data-parallel over 8 NeuronCores (batch sharded 8192/core,
weights replicated). Inside each core activations are kept feature-major
([128 feature partitions, columns = sample-tokens]) so every linear is a
single PE matmul with no transposes.  Per-column statistics (LayerNorm
mean/var, softmax-over-2-tokens) are produced with ones/selector matmuls
on the PE (partition reductions), batched across all tiles of a layer, and
broadcast back to 128 partitions with replicate matmuls.  rsqrt for LN is
computed on the Vector engine with the bit-trick seed + 2 Newton steps so
the Scalar engine stays on a single activation-table set
(gelu_and_others: gelu + tanh + square + copy).  Matmuls run as float32r
(full-rate fp32 mode, valid at N>=256).  Softmax over the 2 keys is
sigmoid via tanh: softmax weight a0 = 0.5 + 0.5*tanh((s0-s1)/2), and the
attention output is o = 0.5*(v0+v1) + 0.5*tanh(d/2)*(v0-v1); both 0.5
factors are folded into the out-projection weights on the host.
"""

import numpy as np

L, T, D, H, NC_CLS = 4, 2, 128, 8, 7
DH = D // H
DFF = 4 * D
EPS = 1e-5
N_CORES = 8
B_FULL = 65536
B_CORE = B_FULL // N_CORES  # 8192
SAMP_PER_TILE = 256         # 256 samples -> 512 columns per tile
COLS = SAMP_PER_TILE * T    # 512

_CACHE = {}


def _build(b_core):
    import concourse.bacc as bacc
    import concourse.tile as tile
    import concourse.mybir as mybir
    from concourse import bass

    f32 = mybir.dt.float32
    f32r = mybir.dt.float32r
    i32 = mybir.dt.int32
    AF = mybir.ActivationFunctionType
    OP = mybir.AluOpType

    n_tiles = b_core // SAMP_PER_TILE
    assert n_tiles <= 32  # selector matrices sized for <=32 tiles

    nc = bacc.Bacc(
        "TRN2", target_bir_lowering=False, debug=False, num_devices=N_CORES
    )

    def din(name, shape, dt=None):
        return nc.dram_tensor(name, shape, dt or f32, kind="ExternalInput").ap()

    x_d = din("x", (b_core, T * D))
    wproj_d = din("wproj", (T * D, T * D), f32r)       # token_proj_w.T  [fin, fout]
    wqkv_d = din("wqkv", (L, D, 3 * D), f32r)          # qkv_w[l].T
    wout_d = din("wout", (L, D, D), f32r)              # 0.5 * out_w[l].T
    wff1_d = din("wff1", (L, D, DFF), f32r)            # ff1_w[l].T
    wff2_d = din("wff2", (L, DFF, D), f32r)            # ff2_w[l].T
    wcls_d = din("wcls", (D, NC_CLS), f32r)            # cls_w.T
    btok_d = din("btok", (D, T))                 # pos_emb (+token_proj_b) [d, t]
    zsel_d = din("zsel", (3, D, 2 * D), f32r)          # scatter selectors 1/128,1/256,1/512
    rsel_d = din("rsel", (32, 32 * D), f32r)           # replicate selectors
    bhead_d = din("bhead", (D, H), f32r)               # 0.125 block-ones
    bbcast_d = din("bbcast", (H, D), f32r)             # head -> partitions block-ones
    ident_d = din("ident", (D, D))               # identity for PE transpose
    out_d = nc.dram_tensor("out", (b_core, NC_CLS), f32, kind="ExternalOutput").ap()

    def mm(out, lhsT, rhs, start=True, stop=True):
        nc.tensor.matmul(out, lhsT.bitcast(f32r), rhs.bitcast(f32r),
                         start=start, stop=stop)

    def bcast_free(ap, n, axis=1):
        """Insert a stride-0 axis of size n at `axis` into a 2D AP."""
        return bass.AP(tensor=ap.tensor, offset=ap.offset,
                       ap=ap.ap[:axis] + [[0, n]] + ap.ap[axis:])

    with tile.TileContext(nc) as tc:
        with (
            tc.tile_pool(name="wpool", bufs=1) as wp,
            tc.tile_pool(name="resid", bufs=1) as rp,
            tc.tile_pool(name="stats", bufs=4) as stp,
            tc.tile_pool(name="stats1", bufs=1) as stp1,
            tc.tile_pool(name="work", bufs=2) as wk,
            tc.tile_pool(name="xin", bufs=2) as xp,
            tc.tile_pool(name="pstat", bufs=1, space="PSUM") as pstat,
            tc.tile_pool(name="pwork", bufs=6, space="PSUM") as pw,
        ):
            # ---- load weights/constants into SBUF (resident) ----
            wproj = wp.tile([D, 2, 2, D], f32r)   # [fin_p, fin_chunk, tok, fout]
            nc.sync.dma_start(
                out=wproj,
                in_=wproj_d.rearrange("(c p) (t d) -> p c t d", p=D, t=T))
            wqkv = wp.tile([D, L, 3 * D], f32r)
            nc.sync.dma_start(out=wqkv, in_=wqkv_d.rearrange("l p f -> p l f"))
            wout = wp.tile([D, L, D], f32r)
            nc.sync.dma_start(out=wout, in_=wout_d.rearrange("l p f -> p l f"))
            wff1 = wp.tile([D, L, DFF], f32r)
            nc.sync.dma_start(out=wff1, in_=wff1_d.rearrange("l p f -> p l f"))
            wff2 = wp.tile([D, L, 4, D], f32r)    # [fin_in_chunk, l, chunk, fout]
            nc.sync.dma_start(
                out=wff2, in_=wff2_d.rearrange("l (c p) f -> p l c f", p=D))
            wcls = wp.tile([D, NC_CLS], f32r)
            nc.sync.dma_start(out=wcls, in_=wcls_d)
            btok = wp.tile([D, T], f32)
            nc.sync.dma_start(out=btok, in_=btok_d)
            zsel = wp.tile([D, 3, 2 * D], f32r)
            nc.sync.dma_start(out=zsel, in_=zsel_d.rearrange("z p f -> p z f"))
            rsel = wp.tile([32, 32 * D], f32r)
            nc.sync.dma_start(out=rsel, in_=rsel_d)
            bhead = wp.tile([D, H], f32r)
            nc.sync.dma_start(out=bhead, in_=bhead_d)
            bbcast = wp.tile([H, D], f32r)
            nc.sync.dma_start(out=bbcast, in_=bbcast_d)
            ident = wp.tile([D, D], f32)
            nc.sync.dma_start(out=ident, in_=ident_d)
            magic = wp.tile([32, 1], i32)
            nc.vector.memset(magic, 0x5F3759DF)

            # residual stream, feature-major [d, tile, tok, samp]
            tok = rp.tile([D, n_tiles, T, SAMP_PER_TILE], f32r)

            def zslice(z, i):
                # selector lhsT whose only non-zero column is column i
                return zsel[:, z, D - i: 2 * D - i]

            def rslice(i, gs):
                return rsel[:gs, i * D:(i + 1) * D]

            def ln_chain(s1, s2, nt, ncols=COLS):
                """Stats chain: mean rows in s1[0:nt], E[x^2] rows in s2[0:nt]
                (both PSUM). Returns sbuf (mean, rstd) [32, ncols]."""
                nt_sl = slice(0, nt)
                mean = stp.tile([32, COLS], f32r, tag="mean")
                nc.scalar.copy(mean[:nt, :ncols], s1[:nt, :ncols])
                u = stp1.tile([32, COLS], f32, tag="u")
                # u = E[x^2] - mean^2
                m2 = stp1.tile([32, COLS], f32, tag="m2")
                nc.vector.tensor_tensor(out=m2[:nt, :ncols],
                                        in0=mean[:nt, :ncols],
                                        in1=mean[:nt, :ncols], op=OP.mult)
                nc.vector.tensor_tensor(out=u[:nt, :ncols],
                                        in0=s2[:nt, :ncols],
                                        in1=m2[:nt, :ncols], op=OP.subtract)
                nc.vector.tensor_scalar(out=u[:nt, :ncols],
                                        in0=u[:nt, :ncols], scalar1=EPS,
                                        scalar2=None, op0=OP.add)
                # quake rsqrt + 2 Newton iterations
                y = stp.tile([32, COLS], i32, tag="y")
                nc.vector.tensor_scalar(out=y[:nt, :ncols],
                                        in0=u.bitcast(i32)[:nt, :ncols],
                                        scalar1=1, scalar2=None,
                                        op0=OP.logical_shift_right)
                nc.vector.tensor_tensor(
                    out=y[:nt, :ncols],
                    in0=bcast_free(magic[:nt, 0:1], ncols, axis=1),
                    in1=y[:nt, :ncols], op=OP.subtract)
                yf = y.bitcast(f32)
                t1 = stp1.tile([32, COLS], f32, tag="t1")
                rstd = stp.tile([32, COLS], f32r, tag="rstd")
                for it in range(2):
                    nc.scalar.activation(t1[:nt, :ncols], yf[:nt, :ncols],
                                         AF.Square)
                    nc.vector.tensor_tensor(out=t1[:nt, :ncols],
                                            in0=u[:nt, :ncols],
                                            in1=t1[:nt, :ncols], op=OP.mult)
                    nc.vector.tensor_scalar(out=t1[:nt, :ncols],
                                            in0=t1[:nt, :ncols],
                                            scalar1=-0.5, scalar2=1.5,
                                            op0=OP.mult, op1=OP.add)
                    dst = rstd if it == 1 else y.bitcast(f32)
                    nc.vector.tensor_tensor(out=dst[:nt, :ncols],
                                            in0=yf[:nt, :ncols],
                                            in1=t1[:nt, :ncols], op=OP.mult)
                return mean, rstd

            def normalize(j, gs, dst, src_ap, mean, rstd, ncols=COLS):
                """dst[...] = (src - repl(mean_j)) * repl(rstd_j)"""
                mb = pw.tile([D, ncols], f32, tag="pwork")
                mm(mb, rslice(j, gs), mean[:gs, :ncols])
                rb = pw.tile([D, ncols], f32, tag="pwork")
                mm(rb, rslice(j, gs), rstd[:gs, :ncols])
                cen = wk.tile([D, ncols], f32, tag="cen")
                nc.vector.tensor_tensor(out=cen, in0=src_ap, in1=mb,
                                        op=OP.subtract)
                nc.vector.tensor_tensor(out=dst, in0=cen, in1=rb, op=OP.mult)

            # ============ phase 0: token projection ============
            s1p = pstat.tile([D, COLS], f32, tag="s1")
            s2p = pstat.tile([D, COLS], f32, tag="s2")
            for i in range(n_tiles):
                xbm = xp.tile([D, 2, T * D], f32, tag="xbm")  # [samp_p, sc, feat]
                nc.sync.dma_start(
                    out=xbm,
                    in_=x_d[i * SAMP_PER_TILE:(i + 1) * SAMP_PER_TILE, :]
                    .rearrange("(sc p) f -> p sc f", p=D))
                xt_ps = pw.tile([D, 2, SAMP_PER_TILE], f32, tag="pwork")
                for fc in range(2):
                    for sc in range(2):
                        nc.tensor.transpose(
                            xt_ps[:, fc, sc * D:(sc + 1) * D],
                            xbm[:, sc, fc * D:(fc + 1) * D], ident)
                xt = xp.tile([D, 2, SAMP_PER_TILE], f32r, tag="xtsb")
                nc.scalar.copy(xt, xt_ps)
                tk_ps = pw.tile([D, T, SAMP_PER_TILE], f32, tag="pwork")
                for t in range(T):
                    for fc in range(2):
                        mm(tk_ps[:, t, :], wproj[:, fc, t, :], xt[:, fc, :],
                           start=(fc == 0), stop=(fc == 1))
                for t in range(T):
                    nc.scalar.activation(tok[:, i, t, :], tk_ps[:, t, :],
                                         AF.Identity, bias=btok[:, t:t + 1])

            # ============ transformer layers ============
            # Tiles are processed in groups; each LN-stats chain only fences
            # its own group, so pass A of group g+1 pipelines with pass B of
            # group g.
            GROUP = min(16, n_tiles)
            n_groups = (n_tiles + GROUP - 1) // GROUP
            groups = [list(range(g * GROUP, min((g + 1) * GROUP, n_tiles)))
                      for g in range(n_groups)]

            def emit_passA(lyr, tiles, prev_stats):
                gs = len(tiles)
                s1p = pstat.tile([D, COLS], f32, tag="s1")
                s2p = pstat.tile([D, COLS], f32, tag="s2")
                for j, i in enumerate(tiles):
                    tki = tok[:, i, :, :]
                    tkf = tki.rearrange("p t s -> p (t s)")
                    if prev_stats is not None:
                        normalize(j, gs, tkf, tkf, *prev_stats)
                    q_ps = pw.tile([D, T, SAMP_PER_TILE], f32, tag="pwork")
                    k_ps = pw.tile([D, T, SAMP_PER_TILE], f32, tag="pwork")
                    v_ps = pw.tile([D, T, SAMP_PER_TILE], f32, tag="pwork")
                    qf = q_ps.rearrange("p t s -> p (t s)")
                    mm(qf, wqkv[:, lyr, 0:D], tkf)
                    mm(k_ps.rearrange("p t s -> p (t s)"),
                       wqkv[:, lyr, D:2 * D], tkf)
                    mm(v_ps.rearrange("p t s -> p (t s)"),
                       wqkv[:, lyr, 2 * D:3 * D], tkf)
                    k1 = wk.tile([D, SAMP_PER_TILE], f32, tag="k1")
                    nc.scalar.copy(k1, k_ps[:, 1, :])
                    v1 = wk.tile([D, SAMP_PER_TILE], f32, tag="v1")
                    nc.scalar.copy(v1, v_ps[:, 1, :])
                    kd = wk.tile([D, SAMP_PER_TILE], f32, tag="kd")
                    nc.vector.tensor_tensor(out=kd, in0=k_ps[:, 0, :],
                                            in1=k1, op=OP.subtract)
                    qd = wk.tile([D, T, SAMP_PER_TILE], f32r, tag="qd")
                    nc.vector.tensor_tensor(out=qd, in0=q_ps,
                                            in1=bcast_free(kd, T), op=OP.mult)
                    sv = wk.tile([D, SAMP_PER_TILE], f32r, tag="sv")
                    nc.vector.tensor_tensor(out=sv, in0=v_ps[:, 0, :],
                                            in1=v1, op=OP.add)
                    dv = wk.tile([D, SAMP_PER_TILE], f32, tag="dv")
                    nc.vector.tensor_tensor(out=dv, in0=v_ps[:, 0, :],
                                            in1=v1, op=OP.subtract)
                    dtb_ps = pw.tile([D, T, SAMP_PER_TILE], f32, tag="pwork")
                    d_ps = dtb_ps.rearrange("p t s -> p (t s)")[:H, :]
                    mm(d_ps, bhead, qd.rearrange("p t s -> p (t s)"))
                    th = wk.tile([H, COLS], f32r, tag="th")
                    nc.scalar.activation(th, d_ps, AF.Tanh)
                    tb_ps = dtb_ps
                    mm(tb_ps.rearrange("p t s -> p (t s)"), bbcast, th)
                    opre = wk.tile([D, T, SAMP_PER_TILE], f32r, tag="opre")
                    nc.vector.tensor_tensor(out=opre, in0=tb_ps,
                                            in1=bcast_free(dv, T), op=OP.mult)
                    o_ps = pw.tile([D, COLS], f32, tag="pwork")
                    mm(o_ps, wout[:, lyr, :],
                       opre.rearrange("p t s -> p (t s)"),
                       start=True, stop=False)
                    mm(o_ps, wout[:, lyr, :], bcast_free(sv, T),
                       start=False, stop=True)
                    nc.vector.tensor_tensor(out=tkf, in0=tkf, in1=o_ps,
                                            op=OP.add)
                    sq = wk.tile([D, COLS], f32r, tag="sq")
                    nc.scalar.activation(sq, tkf, AF.Square)
                    mm(s1p, zslice(0, j), tkf,
                       start=(j == 0), stop=(j == gs - 1))
                    mm(s2p, zslice(0, j), sq,
                       start=(j == 0), stop=(j == gs - 1))
                return ln_chain(s1p, s2p, gs)

            def emit_passB(lyr, tiles, stats1):
                gs = len(tiles)
                s1p = pstat.tile([D, COLS], f32, tag="s1")
                s2p = pstat.tile([D, COLS], f32, tag="s2")
                for j, i in enumerate(tiles):
                    tkf = tok[:, i, :, :].rearrange("p t s -> p (t s)")
                    normalize(j, gs, tkf, tkf, *stats1)
                    h = wk.tile([D, 4, COLS], f32r, tag="h_sb")
                    for c in range(4):
                        h_ps = pw.tile([D, COLS], f32, tag="pwork")
                        mm(h_ps, wff1[:, lyr, c * D:(c + 1) * D], tkf)
                        nc.scalar.activation(h[:, c, :], h_ps, AF.Gelu)
                    f_ps = pw.tile([D, COLS], f32, tag="pwork")
                    for c in range(4):
                        mm(f_ps, wff2[:, lyr, c, :], h[:, c, :],
                           start=(c == 0), stop=(c == 3))
                    nc.vector.tensor_tensor(out=tkf, in0=tkf, in1=f_ps,
                                            op=OP.add)
                    sq = wk.tile([D, COLS], f32r, tag="sq")
                    nc.scalar.activation(sq, tkf, AF.Square)
                    mm(s1p, zslice(0, j), tkf,
                       start=(j == 0), stop=(j == gs - 1))
                    mm(s2p, zslice(0, j), sq,
                       start=(j == 0), stop=(j == gs - 1))
                return ln_chain(s1p, s2p, gs)

            def emit_lnpass(tiles, prev_stats):
                gs = len(tiles)
                s1p = pstat.tile([D, COLS], f32, tag="s1")
                s2p = pstat.tile([D, COLS], f32, tag="s2")
                for j, i in enumerate(tiles):
                    tkf = tok[:, i, :, :].rearrange("p t s -> p (t s)")
                    normalize(j, gs, tkf, tkf, *prev_stats)
                    sq = wk.tile([D, COLS], f32r, tag="sq")
                    nc.scalar.activation(sq, tkf, AF.Square)
                    mm(s1p, zslice(0, j), tkf,
                       start=(j == 0), stop=(j == gs - 1))
                    mm(s2p, zslice(0, j), sq,
                       start=(j == 0), stop=(j == gs - 1))
                return ln_chain(s1p, s2p, gs)

            def emit_H2(tiles, statsf):
                gs = len(tiles)
                s1p = pstat.tile([D, COLS], f32, tag="s1")
                s2p = pstat.tile([D, COLS], f32, tag="s2")
                for j, i in enumerate(tiles):
                    tki = tok[:, i, :, :]
                    tkf = tki.rearrange("p t s -> p (t s)")
                    normalize(j, gs, tkf, tkf, *statsf)
                    nc.vector.tensor_tensor(out=tki[:, 0, :],
                                            in0=tki[:, 0, :],
                                            in1=tki[:, 1, :], op=OP.add)
                    sq = wk.tile([D, SAMP_PER_TILE], f32r, tag="sqh")
                    nc.scalar.activation(sq, tki[:, 0, :], AF.Square)
                    mm(s1p[:, :SAMP_PER_TILE], zslice(1, j), tki[:, 0, :],
                       start=(j == 0), stop=(j == gs - 1))
                    mm(s2p[:, :SAMP_PER_TILE], zslice(2, j), sq,
                       start=(j == 0), stop=(j == gs - 1))
                return ln_chain(s1p, s2p, gs, ncols=SAMP_PER_TILE)

            stats_p = [None] * n_groups
            for lyr in range(L):
                stats1 = [None] * n_groups
                for g in range(n_groups):
                    stats1[g] = emit_passA(lyr, groups[g], stats_p[g])
                for g in range(n_groups):
                    stats_p[g] = emit_passB(lyr, groups[g], stats1[g])

            # ============ head ============
            statsf = [None] * n_groups
            for g in range(n_groups):
                statsf[g] = emit_lnpass(groups[g], stats_p[g])
            statsc = [None] * n_groups
            for g in range(n_groups):
                statsc[g] = emit_H2(groups[g], statsf[g])

            # H3: cls_ln normalize + gelu + classifier + output
            for g in range(n_groups):
              gs = len(groups[g])
              meanc, rstdc = statsc[g]
              for j, i in enumerate(groups[g]):
                p2 = tok[:, i, 0, :]
                mb = pw.tile([D, SAMP_PER_TILE], f32, tag="pwork")
                mm(mb, rslice(j, gs), meanc[:gs, :SAMP_PER_TILE])
                rb = pw.tile([D, SAMP_PER_TILE], f32, tag="pwork")
                mm(rb, rslice(j, gs), rstdc[:gs, :SAMP_PER_TILE])
                cen = wk.tile([D, SAMP_PER_TILE], f32, tag="cen")
                nc.vector.scalar_tensor_tensor(
                    out=cen, in0=p2, scalar=0.5, in1=mb,
                    op0=OP.mult, op1=OP.subtract)
                xh = wk.tile([D, SAMP_PER_TILE], f32, tag="xh")
                nc.vector.tensor_tensor(out=xh, in0=cen, in1=rb, op=OP.mult)
                gl = wk.tile([D, SAMP_PER_TILE], f32r, tag="g")
                nc.scalar.activation(gl, xh, AF.Gelu)
                cls_ps = pw.tile([NC_CLS, SAMP_PER_TILE], f32, tag="pwork")
                mm(cls_ps, wcls, gl)
                cls_sb = wk.tile([NC_CLS, SAMP_PER_TILE], f32, tag="clssb")
                nc.scalar.copy(cls_sb, cls_ps)
                tr_ps = pw.tile([D, 2, NC_CLS], f32, tag="pwork")
                for sc in range(2):
                    nc.tensor.transpose(tr_ps[:, sc, :],
                                        cls_sb[:, sc * D:(sc + 1) * D],
                                        ident[:NC_CLS, :NC_CLS])
                obm = wk.tile([D, 2, NC_CLS], f32, tag="obm")
                nc.scalar.copy(obm, tr_ps)
                nc.sync.dma_start(
                    out=out_d[i * SAMP_PER_TILE:(i + 1) * SAMP_PER_TILE, :]
                    .rearrange("(sc p) c -> p sc c", p=D),
                    in_=obm)

    nc.compile()
    return nc


def _prep_weights(inputs):
    w = {}
    w["wproj"] = np.ascontiguousarray(inputs["token_proj_w"].T)
    w["wqkv"] = np.ascontiguousarray(inputs["qkv_w"].transpose(0, 2, 1))
    w["wout"] = np.ascontiguousarray(0.5 * inputs["out_w"].transpose(0, 2, 1))
    w["wff1"] = np.ascontiguousarray(inputs["ff1_w"].transpose(0, 2, 1))
    w["wff2"] = np.ascontiguousarray(inputs["ff2_w"].transpose(0, 2, 1))
    w["wcls"] = np.ascontiguousarray(inputs["cls_w"].T)
    w["btok"] = np.ascontiguousarray(
        inputs["pos_emb"][0].T
        + inputs["token_proj_b"].reshape(T, D).T)
    zsel = np.zeros((3, D, 2 * D), dtype=np.float32)
    zsel[0, :, D] = 1.0 / 128
    zsel[1, :, D] = 1.0 / 256
    zsel[2, :, D] = 1.0 / 512
    w["zsel"] = zsel
    rsel = np.zeros((32, 32 * D), dtype=np.float32)
    for i in range(32):
        rsel[i, i * D:(i + 1) * D] = 1.0
    w["rsel"] = rsel
    bhead = np.zeros((D, H), dtype=np.float32)
    for h in range(H):
        bhead[h * DH:(h + 1) * DH, h] = 0.125
    w["bhead"] = bhead
    w["bbcast"] = np.ascontiguousarray(bhead.T != 0).astype(np.float32)
    w["ident"] = np.eye(D, dtype=np.float32)

    # Unused-by-construction inputs (all zeros / ones in this model family);
    # verify that so silently ignoring them is sound.
    for name in ("qkv_b", "out_b", "ff1_b", "ff2_b", "cls_b"):
        assert not np.any(inputs[name]), f"{name} expected to be all zeros"
    for name in ("ln1_w", "ln2_w", "lnf_w", "cls_ln_w"):
        assert np.all(inputs[name] == 1.0), f"{name} expected to be all ones"
    for name in ("ln1_b", "ln2_b", "lnf_b", "cls_ln_b"):
        assert not np.any(inputs[name]), f"{name} expected to be all zeros"
    return {k: np.ascontiguousarray(v, dtype=np.float32) for k, v in w.items()}


def kernel(**inputs):
    from concourse.bass_utils import run_bass_kernel_spmd

    x = np.asarray(inputs["x"], dtype=np.float32).reshape(B_FULL, T * D)
    if "nc" not in _CACHE:
        _CACHE["nc"] = _build(B_CORE)
    nc = _CACHE["nc"]

    w = _prep_weights(inputs)
    in_maps = []
    for c in range(N_CORES):
        m = dict(w)
        m["x"] = np.ascontiguousarray(x[c * B_CORE:(c + 1) * B_CORE])
        in_maps.append(m)

    res = run_bass_kernel_spmd(nc, in_maps, core_ids=list(range(N_CORES)))
    out = np.concatenate([r["out"] for r in res.results], axis=0)
    return out.astype(np.float32)



# revision 11
# speedup vs baseline: 1.1563x; 1.1563x over previous
"""Trainium2 Bass kernel for nn_CategoryHead (tiny 4-layer post-norm
transformer classifier head over B=65536 samples, T=2 tokens, D=128).

Strategy: pure data-parallel over 8 NeuronCores (batch sharded 8192/core,
weights replicated). Feature-major activations ([128 feature partitions,
columns = sample-tokens]); the residual stream and all stream-path matmuls
stay fp32 (float32r full-rate mode), while attention internals (k-diff,
v-diff, tanh weights, o-premix) run bf16 where quantization error is
negligible.  Per-column LN stats are produced with ones-selector matmuls on
the PE (deferred two tiles behind the main per-tile chain so the PE queue
never head-of-line blocks on them), rsqrt via quake seed + 1 Newton step,
stats broadcast back with replicate matmuls.  Attention over T=2 uses
softmax-as-sigmoid: with xs=x0+x1, xd=x0-x1, the output is
o = 0.5*Wo^T Wv^T xs + Wo^T(tanh(d/2)_heads * 0.5*Wv^T xd); the xs-term
uses a host-fused weight (Wvo) accumulated straight into the output PSUM,
and the residual add rides the PE as an identity matmul into the same
accumulation group.  Squares for the variance stats run on the otherwise
idle GPSIMD engine; gelu is batched [128, 1024] on the Scalar engine.
"""

import numpy as np

L, T, D, H, NC_CLS = 4, 2, 128, 8, 7
DH = D // H
DFF = 4 * D
EPS = 1e-5
N_CORES = 8
B_FULL = 65536
B_CORE = B_FULL // N_CORES  # 8192
SAMP_PER_TILE = 256         # 256 samples -> 512 columns per tile
COLS = SAMP_PER_TILE * T    # 512

_CACHE = {}


def _build(b_core):
    import concourse.bacc as bacc
    import concourse.tile as tile
    import concourse.mybir as mybir
    from concourse import bass

    f32 = mybir.dt.float32
    f32r = mybir.dt.float32r
    bf16 = mybir.dt.bfloat16
    i32 = mybir.dt.int32
    AF = mybir.ActivationFunctionType
    OP = mybir.AluOpType

    n_tiles = b_core // SAMP_PER_TILE
    assert n_tiles <= 32

    nc = bacc.Bacc(
        "TRN2", target_bir_lowering=False, debug=False, num_devices=N_CORES
    )

    def din(name, shape, dt=f32):
        return nc.dram_tensor(name, shape, dt, kind="ExternalInput").ap()

    x_d = din("x", (D, 2, b_core), f32r)                 # feature-major chunks of x
    wproj_d = din("wproj", (D, 2, T, D), f32r)           # [fin_p, fin_chunk, tok, fout]
    wqkv_d = din("wqkv", (L, D, 3 * D), f32r)            # qkv_w[l].T
    wout_d = din("wout", (L, D, D), bf16)          # 0.5 * out_w[l].T
    wvo_d = din("wvo", (L, D, D), f32r)                  # 0.5 * (v_w out_w)[l].T fused
    wff1_d = din("wff1", (L, D, DFF), f32r)              # ff1_w[l].T
    wff2_d = din("wff2", (L, DFF, D), f32r)              # ff2_w[l].T
    wcls_d = din("wcls", (D, NC_CLS), bf16)        # cls_w.T
    btok_d = din("btok", (D, T))                   # pos_emb [d, t]
    zsel_d = din("zsel", (3, D, 2 * D), f32r)            # scatter sel 1/128,1/256,1/512
    rsel_d = din("rsel", (32, 32 * D), f32r)             # replicate selectors
    bhead_d = din("bhead", (D, H), bf16)           # 0.125 block-ones
    bbcast_d = din("bbcast", (H, D), bf16)         # head -> partitions block-ones
    ident_d = din("ident", (D, D), f32r)
    identf_d = din("identf", (32, 32))                 # identity for resid mms
    out_d = nc.dram_tensor("out", (b_core, NC_CLS), f32, kind="ExternalOutput").ap()

    def mm(out, lhsT, rhs, start=True, stop=True):
        nc.tensor.matmul(out, lhsT.bitcast(f32r), rhs.bitcast(f32r),
                         start=start, stop=stop)

    mmb = nc.tensor.matmul  # bf16 matmul (attention internals)

    def bcast_free(ap, n, axis=1):
        """Insert a stride-0 axis of size n at `axis` into a 2D AP."""
        return bass.AP(tensor=ap.tensor, offset=ap.offset,
                       ap=ap.ap[:axis] + [[0, n]] + ap.ap[axis:])

    with tile.TileContext(nc) as tc:
        with (
            tc.tile_pool(name="wpool", bufs=1) as wp,
            tc.tile_pool(name="resid", bufs=1) as rp,
            tc.tile_pool(name="stats", bufs=2) as stp,
            tc.tile_pool(name="stats1", bufs=2) as stp1,
            tc.tile_pool(name="work", bufs=3) as wk,
            tc.tile_pool(name="hpool", bufs=2) as hp,
            tc.tile_pool(name="sqpool", bufs=5) as sqp,
            tc.tile_pool(name="xin", bufs=3) as xp,
            tc.tile_pool(name="pstat", bufs=1, space="PSUM") as pstat,
            tc.tile_pool(name="pwork", bufs=4, space="PSUM") as pw,
            tc.tile_pool(name="ph", bufs=1, space="PSUM") as ph,
        ):
            # ---- load weights/constants into SBUF (resident) ----
            wproj = wp.tile([D, 2, T, D], f32r)
            nc.sync.dma_start(out=wproj, in_=wproj_d)
            wqkv = wp.tile([D, L, 3 * D], f32r)
            nc.sync.dma_start(out=wqkv, in_=wqkv_d.rearrange("l p f -> p l f"))
            wout = wp.tile([D, L, D], bf16)
            nc.sync.dma_start(out=wout, in_=wout_d.rearrange("l p f -> p l f"))
            wvo = wp.tile([D, L, D], f32r)
            nc.sync.dma_start(out=wvo, in_=wvo_d.rearrange("l p f -> p l f"))
            wff1 = wp.tile([D, L, DFF], f32r)
            nc.sync.dma_start(out=wff1, in_=wff1_d.rearrange("l p f -> p l f"))
            wff2 = wp.tile([D, L, 4, D], f32r)
            nc.sync.dma_start(
                out=wff2, in_=wff2_d.rearrange("l (c p) f -> p l c f", p=D))
            wcls = wp.tile([D, NC_CLS], bf16)
            nc.sync.dma_start(out=wcls, in_=wcls_d)
            btok = wp.tile([D, T], f32)
            nc.sync.dma_start(out=btok, in_=btok_d)
            zsel = wp.tile([D, 3, 2 * D], f32r)
            nc.sync.dma_start(out=zsel, in_=zsel_d.rearrange("z p f -> p z f"))
            rsel = wp.tile([32, 32 * D], f32r)
            nc.sync.dma_start(out=rsel, in_=rsel_d)
            bhead = wp.tile([D, H], bf16)
            nc.sync.dma_start(out=bhead, in_=bhead_d)
            bbcast = wp.tile([H, D], bf16)
            nc.sync.dma_start(out=bbcast, in_=bbcast_d)
            ident = wp.tile([D, D], f32r)
            nc.sync.dma_start(out=ident, in_=ident_d)
            identf = wp.tile([32, 32], f32)
            nc.sync.dma_start(out=identf, in_=identf_d)
            magic = wp.tile([32, 1], i32)
            nc.vector.memset(magic, 0x5F3759DF)

            # residual stream, feature-major [d, tile, tok, samp], fp32
            tok = rp.tile([D, n_tiles, T, SAMP_PER_TILE], f32r)

            def zslice(z, i):
                return zsel[:, z, D - i: 2 * D - i]

            def rslice(i, gs):
                return rsel[:gs, i * D:(i + 1) * D]

            def ln_chain(s1, s2, nt, ncols=COLS):
                """Stats chain from PSUM mean rows s1[0:nt] and E[x^2] rows
                s2[0:nt].  Returns sbuf f32 (negmean, rstd) [32, ncols]."""
                negmean = stp.tile([32, COLS], f32r, tag="negmean")
                nc.scalar.activation(negmean[:nt, :ncols], s1[:nt, :ncols],
                                     AF.Identity, scale=-1.0)
                m2 = stp1.tile([32, COLS], f32, tag="m2")
                nc.vector.tensor_tensor(out=m2[:nt, :ncols],
                                        in0=negmean[:nt, :ncols],
                                        in1=negmean[:nt, :ncols], op=OP.mult)
                # u = (s2 + EPS) - m^2
                u = stp1.tile([32, COLS], f32, tag="u")
                nc.vector.scalar_tensor_tensor(
                    out=u[:nt, :ncols], in0=s2[:nt, :ncols], scalar=EPS,
                    in1=m2[:nt, :ncols], op0=OP.add, op1=OP.subtract)
                # quake rsqrt + 1 Newton iteration (fp32)
                y = stp.tile([32, COLS], i32, tag="y")
                nc.vector.tensor_scalar(out=y[:nt, :ncols],
                                        in0=u.bitcast(i32)[:nt, :ncols],
                                        scalar1=1, scalar2=None,
                                        op0=OP.logical_shift_right)
                nc.vector.tensor_tensor(
                    out=y[:nt, :ncols],
                    in0=bcast_free(magic[:nt, 0:1], ncols, axis=1),
                    in1=y[:nt, :ncols], op=OP.subtract)
                yf = y.bitcast(f32)
                t1 = stp1.tile([32, COLS], f32, tag="t1")
                nc.scalar.activation(t1[:nt, :ncols], yf[:nt, :ncols],
                                     AF.Square)
                nc.vector.tensor_tensor(out=t1[:nt, :ncols],
                                        in0=u[:nt, :ncols],
                                        in1=t1[:nt, :ncols], op=OP.mult)
                nc.vector.tensor_scalar(out=t1[:nt, :ncols],
                                        in0=t1[:nt, :ncols],
                                        scalar1=-0.5, scalar2=1.5,
                                        op0=OP.mult, op1=OP.add)
                rstd = stp.tile([32, COLS], f32r, tag="rstd")
                nc.vector.tensor_tensor(out=rstd[:nt, :ncols],
                                        in0=yf[:nt, :ncols],
                                        in1=t1[:nt, :ncols], op=OP.mult)
                return negmean, rstd

            def normalize(j, gs, dst, src_ap, negmean, rstd, ncols=COLS):
                """dst[...] = (src + repl(negmean_j)) * repl(rstd_j)"""
                mb = pw.tile([D, ncols], f32, tag="pwork")
                mm(mb, rslice(j, gs), negmean[:gs, :ncols])
                rb = pw.tile([D, ncols], f32, tag="pwork")
                mm(rb, rslice(j, gs), rstd[:gs, :ncols])
                cen = wk.tile([D, ncols], f32, tag="cen")
                nc.vector.tensor_tensor(out=cen, in0=src_ap, in1=mb, op=OP.add)
                nc.vector.tensor_tensor(out=dst, in0=cen, in1=rb, op=OP.mult)

            # Deferred stats: emit each tile's selector matmuls two tiles
            # late so the PE queue never blocks on the Act copy / GPSIMD
            # square that produce their inputs.
            class StatQ:
                def __init__(self, s1p, s2p, gs, z1=0, z2=0, ncols=COLS):
                    self.s1p, self.s2p, self.gs = s1p, s2p, gs
                    self.z1, self.z2, self.ncols = z1, z2, ncols
                    self.pend = []

                def push(self, j, src, sq):
                    self.pend.append((j, src, sq))
                    if len(self.pend) > 2:
                        self.emit_one()

                def emit_one(self):
                    j, src, sq = self.pend.pop(0)
                    g = self.gs
                    mm(self.s1p[:, :self.ncols], zslice(self.z1, j), src,
                       start=(j == 0), stop=(j == g - 1))
                    mm(self.s2p[:, :self.ncols], zslice(self.z2, j), sq,
                       start=(j == 0), stop=(j == g - 1))

                def flush(self):
                    while self.pend:
                        self.emit_one()

            # ============ phase 0: token projection ============
            for i in range(n_tiles):
                xt = xp.tile([D, 2, SAMP_PER_TILE], f32r, tag="xt")
                nc.sync.dma_start(
                    out=xt,
                    in_=x_d[:, :, i * SAMP_PER_TILE:(i + 1) * SAMP_PER_TILE])
                tk_ps = pw.tile([D, T, SAMP_PER_TILE], f32, tag="pwork")
                for t in range(T):
                    for fc in range(2):
                        mm(tk_ps[:, t, :], wproj[:, fc, t, :], xt[:, fc, :],
                           start=(fc == 0), stop=(fc == 1))
                for t in range(T):
                    nc.scalar.activation(tok[:, i, t, :], tk_ps[:, t, :],
                                         AF.Identity, bias=btok[:, t:t + 1])

            # ============ transformer layers ============
            GROUP = min(16, n_tiles)
            n_groups = (n_tiles + GROUP - 1) // GROUP
            groups = [list(range(g * GROUP, min((g + 1) * GROUP, n_tiles)))
                      for g in range(n_groups)]

            def emit_passA(lyr, tiles, prev_stats):
                gs = len(tiles)
                s1p = pstat.tile([D, COLS], f32, tag="s1")
                s2p = pstat.tile([D, COLS], f32, tag="s2")
                sk = StatQ(s1p, s2p, gs)
                for j, i in enumerate(tiles):
                    tki = tok[:, i, :, :]
                    tkf = tki.rearrange("p t s -> p (t s)")
                    if prev_stats is not None:
                        normalize(j, gs, tkf, tkf, *prev_stats)
                    # xs = x0+x1, xd = x0-x1
                    xs = wk.tile([D, SAMP_PER_TILE], f32r, tag="xs")
                    nc.vector.tensor_tensor(out=xs, in0=tki[:, 0, :],
                                            in1=tki[:, 1, :], op=OP.add)
                    xd = wk.tile([D, SAMP_PER_TILE], f32r, tag="xd")
                    nc.vector.tensor_tensor(out=xd, in0=tki[:, 0, :],
                                            in1=tki[:, 1, :], op=OP.subtract)
                    # q (both tokens), kd = Wk^T xd, dv = Wv^T xd
                    q_ps = pw.tile([D, T, SAMP_PER_TILE], f32, tag="pwork")
                    qf_ps = q_ps.rearrange("p t s -> p (t s)")
                    mm(qf_ps, wqkv[:, lyr, 0:D], tkf)
                    kdv_ps = pw.tile([D, 2, SAMP_PER_TILE], f32, tag="pwork")
                    mm(kdv_ps[:, 0, :], wqkv[:, lyr, D:2 * D], xd)
                    mm(kdv_ps[:, 1, :], wqkv[:, lyr, 2 * D:3 * D], xd)
                    kdv = wk.tile([D, 2, SAMP_PER_TILE], bf16, tag="kdv")
                    nc.scalar.copy(kdv, kdv_ps)
                    # qd = q * bcast(kd); d = headsum(qd)/8; th = tanh(d)
                    qd = wk.tile([D, T, SAMP_PER_TILE], bf16, tag="qd")
                    nc.vector.tensor_tensor(out=qd, in0=q_ps,
                                            in1=bcast_free(kdv[:, 0, :], T),
                                            op=OP.mult)
                    # d/tb reuse q's PSUM bank (q fully consumed by qd)
                    d_ps = qf_ps[:H, :]
                    mmb(d_ps, bhead, qd.rearrange("p t s -> p (t s)"),
                        start=True, stop=True)
                    th = wk.tile([H, COLS], bf16, tag="th")
                    nc.scalar.activation(th, d_ps, AF.Tanh)
                    tb_ps = q_ps
                    mmb(tb_ps.rearrange("p t s -> p (t s)"), bbcast, th,
                        start=True, stop=True)
                    opre = wk.tile([D, T, SAMP_PER_TILE], bf16, tag="opre")
                    nc.vector.tensor_tensor(out=opre, in0=tb_ps,
                                            in1=bcast_free(kdv[:, 1, :], T),
                                            op=OP.mult)
                    # o = Wvo^T xs (both halves) + Wout^T opre + tok  (resid
                    # add rides the PE as an identity-matmul accumulation)
                    o_ps = pw.tile([D, T, SAMP_PER_TILE], f32, tag="pwork")
                    of = o_ps.rearrange("p t s -> p (t s)")
                    for t in range(T):
                        mm(o_ps[:, t, :], wvo[:, lyr, :], xs,
                           start=True, stop=False)
                    mmb(of, wout[:, lyr, :],
                        opre.rearrange("p t s -> p (t s)"),
                        start=False, stop=False)
                    mm(of, ident, tkf, start=False, stop=True)
                    # stream copy PSUM->SBUF (Act), square on GPSIMD
                    nc.scalar.copy(tkf, of)
                    sq = sqp.tile([D, COLS], f32r, tag="sq")
                    nc.gpsimd.tensor_tensor(out=sq, in0=tkf, in1=tkf,
                                            op=OP.mult)
                    sk.push(j, tkf, sq)
                sk.flush()
                return ln_chain(s1p, s2p, gs)

            def emit_passB(lyr, tiles, stats1):
                gs = len(tiles)
                s1p = pstat.tile([D, COLS], f32, tag="s1")
                s2p = pstat.tile([D, COLS], f32, tag="s2")
                sk = StatQ(s1p, s2p, gs)
                for j, i in enumerate(tiles):
                    tkf = tok[:, i, :, :].rearrange("p t s -> p (t s)")
                    normalize(j, gs, tkf, tkf, *stats1)
                    h = hp.tile([D, 4, COLS], f32r, tag="h_sb")
                    h_ps = ph.tile([D, 2, COLS], f32, tag="h_ps")
                    for cp in range(2):
                        for c in range(2):
                            mm(h_ps[:, c, :],
                               wff1[:, lyr, (2 * cp + c) * D:
                                    (2 * cp + c + 1) * D], tkf)
                        nc.scalar.activation(
                            h[:, 2 * cp:2 * cp + 2, :], h_ps, AF.Gelu)
                    f_ps = pw.tile([D, COLS], f32, tag="pwork")
                    for c in range(4):
                        mm(f_ps, wff2[:, lyr, c, :], h[:, c, :],
                           start=(c == 0), stop=False)
                    mm(f_ps, ident, tkf, start=False, stop=True)
                    nc.scalar.copy(tkf, f_ps)
                    sq = sqp.tile([D, COLS], f32r, tag="sq")
                    nc.gpsimd.tensor_tensor(out=sq, in0=tkf, in1=tkf,
                                            op=OP.mult)
                    sk.push(j, tkf, sq)
                sk.flush()
                return ln_chain(s1p, s2p, gs)

            def emit_lnpass(tiles, prev_stats):
                gs = len(tiles)
                s1p = pstat.tile([D, COLS], f32, tag="s1")
                s2p = pstat.tile([D, COLS], f32, tag="s2")
                sk = StatQ(s1p, s2p, gs)
                for j, i in enumerate(tiles):
                    tkf = tok[:, i, :, :].rearrange("p t s -> p (t s)")
                    normalize(j, gs, tkf, tkf, *prev_stats)
                    sq = sqp.tile([D, COLS], f32r, tag="sq")
                    nc.gpsimd.tensor_tensor(out=sq, in0=tkf, in1=tkf,
                                            op=OP.mult)
                    sk.push(j, tkf, sq)
                sk.flush()
                return ln_chain(s1p, s2p, gs)

            def emit_H2(tiles, statsf):
                gs = len(tiles)
                s1p = pstat.tile([D, COLS], f32, tag="s1")
                s2p = pstat.tile([D, COLS], f32, tag="s2")
                sk = StatQ(s1p, s2p, gs, z1=1, z2=2, ncols=SAMP_PER_TILE)
                for j, i in enumerate(tiles):
                    tki = tok[:, i, :, :]
                    tkf = tki.rearrange("p t s -> p (t s)")
                    normalize(j, gs, tkf, tkf, *statsf)
                    nc.vector.tensor_tensor(out=tki[:, 0, :],
                                            in0=tki[:, 0, :],
                                            in1=tki[:, 1, :], op=OP.add)
                    sq = sqp.tile([D, SAMP_PER_TILE], f32r, tag="sqh")
                    nc.gpsimd.tensor_tensor(out=sq, in0=tki[:, 0, :],
                                            in1=tki[:, 0, :], op=OP.mult)
                    sk.push(j, tki[:, 0, :], sq)
                sk.flush()
                return ln_chain(s1p, s2p, gs, ncols=SAMP_PER_TILE)

            stats_p = [None] * n_groups
            for lyr in range(L):
                stats1 = [None] * n_groups
                for g in range(n_groups):
                    stats1[g] = emit_passA(lyr, groups[g], stats_p[g])
                for g in range(n_groups):
                    stats_p[g] = emit_passB(lyr, groups[g], stats1[g])

            # ============ head ============
            statsf = [None] * n_groups
            for g in range(n_groups):
                statsf[g] = emit_lnpass(groups[g], stats_p[g])
            statsc = [None] * n_groups
            for g in range(n_groups):
                statsc[g] = emit_H2(groups[g], statsf[g])

            # H3: cls_ln normalize + gelu + classifier + output
            for g in range(n_groups):
              gs = len(groups[g])
              negmc, rstdc = statsc[g]
              for j, i in enumerate(groups[g]):
                p2 = tok[:, i, 0, :]
                mb = pw.tile([D, SAMP_PER_TILE], f32, tag="pwork")
                mm(mb, rslice(j, gs), negmc[:gs, :SAMP_PER_TILE])
                rb = pw.tile([D, SAMP_PER_TILE], f32, tag="pwork")
                mm(rb, rslice(j, gs), rstdc[:gs, :SAMP_PER_TILE])
                cen = wk.tile([D, SAMP_PER_TILE], f32, tag="cen")
                nc.vector.scalar_tensor_tensor(
                    out=cen, in0=p2, scalar=0.5, in1=mb,
                    op0=OP.mult, op1=OP.add)
                xh = wk.tile([D, SAMP_PER_TILE], bf16, tag="xh")
                nc.vector.tensor_tensor(out=xh, in0=cen, in1=rb, op=OP.mult)
                gl = wk.tile([D, SAMP_PER_TILE], bf16, tag="g")
                nc.scalar.activation(gl, xh, AF.Gelu)
                cls_ps = pw.tile([NC_CLS, SAMP_PER_TILE], f32, tag="pwork")
                mmb(cls_ps, wcls, gl, start=True, stop=True)
                cls_sb = wk.tile([NC_CLS, SAMP_PER_TILE], f32, tag="clssb")
                nc.scalar.copy(cls_sb, cls_ps)
                tr_ps = pw.tile([D, 2, NC_CLS], f32, tag="pwork")
                for sc in range(2):
                    nc.tensor.transpose(tr_ps[:, sc, :],
                                        cls_sb[:, sc * D:(sc + 1) * D],
                                        identf[:NC_CLS, :NC_CLS])
                obm = wk.tile([D, 2, NC_CLS], f32, tag="obm")
                nc.scalar.copy(obm, tr_ps)
                nc.sync.dma_start(
                    out=out_d[i * SAMP_PER_TILE:(i + 1) * SAMP_PER_TILE, :]
                    .rearrange("(sc p) c -> p sc c", p=D),
                    in_=obm)

    nc.compile()
    return nc


def _to_bf16(a):
    import ml_dtypes
    return np.ascontiguousarray(np.asarray(a, np.float32)).astype(
        ml_dtypes.bfloat16)


def _f32(a):
    return np.ascontiguousarray(np.asarray(a, dtype=np.float32))


def _prep_weights(inputs):
    w = {}
    tp = np.asarray(inputs["token_proj_w"], np.float32).T  # [fin, fout]
    w["wproj"] = _f32(tp.reshape(2, D, T, D).transpose(1, 0, 2, 3))
    qkvT = np.asarray(inputs["qkv_w"], np.float32).transpose(0, 2, 1)
    w["wqkv"] = _f32(qkvT)
    outT = 0.5 * np.asarray(inputs["out_w"], np.float32).transpose(0, 2, 1)
    w["wout"] = _to_bf16(outT)
    # Wvo[l] = (qkv_w[l].T)[:, 2D:3D] @ (0.5*out_w[l].T): x-space -> o-space
    w["wvo"] = _f32(np.einsum('lpv,lvo->lpo', qkvT[:, :, 2 * D:3 * D], outT))
    w["wff1"] = _f32(np.asarray(inputs["ff1_w"], np.float32).transpose(0, 2, 1))
    w["wff2"] = _f32(np.asarray(inputs["ff2_w"], np.float32).transpose(0, 2, 1))
    w["wcls"] = _to_bf16(np.asarray(inputs["cls_w"], np.float32).T)
    w["btok"] = _f32(
        np.asarray(inputs["pos_emb"], np.float32)[0].T
        + np.asarray(inputs["token_proj_b"], np.float32).reshape(T, D).T)
    zsel = np.zeros((3, D, 2 * D), dtype=np.float32)
    zsel[0, :, D] = 1.0 / 128
    zsel[1, :, D] = 1.0 / 256
    zsel[2, :, D] = 1.0 / 512
    w["zsel"] = zsel
    rsel = np.zeros((32, 32 * D), dtype=np.float32)
    for i in range(32):
        rsel[i, i * D:(i + 1) * D] = 1.0
    w["rsel"] = rsel
    bhead = np.zeros((D, H), dtype=np.float32)
    for h in range(H):
        bhead[h * DH:(h + 1) * DH, h] = 0.125
    w["bhead"] = _to_bf16(bhead)
    w["bbcast"] = _to_bf16((bhead.T != 0).astype(np.float32))
    w["ident"] = np.eye(D, dtype=np.float32)
    w["identf"] = np.eye(32, dtype=np.float32)

    # Unused-by-construction inputs (all zeros / ones in this model family);
    # verify that so silently ignoring them is sound.
    for name in ("qkv_b", "out_b", "ff1_b", "ff2_b", "cls_b"):
        assert not np.any(inputs[name]), f"{name} expected to be all zeros"
    for name in ("ln1_w", "ln2_w", "lnf_w", "cls_ln_w"):
        assert np.all(np.asarray(inputs[name]) == 1.0), \
            f"{name} expected to be all ones"
    for name in ("ln1_b", "ln2_b", "lnf_b", "cls_ln_b"):
        assert not np.any(inputs[name]), f"{name} expected to be all zeros"
    return w


def kernel(**inputs):
    from concourse.bass_utils import run_bass_kernel_spmd

    x = np.asarray(inputs["x"], dtype=np.float32).reshape(B_FULL, T * D)
    if "nc" not in _CACHE:
        _CACHE["nc"] = _build(B_CORE)
    nc = _CACHE["nc"]

    w = _prep_weights(inputs)
    in_maps = []
    for c in range(N_CORES):
        xc = x[c * B_CORE:(c + 1) * B_CORE]            # [b, 256]
        xf = xc.reshape(B_CORE, 2, D).transpose(2, 1, 0)  # [D, 2, b]
        m = dict(w)
        m["x"] = _f32(xf)
        in_maps.append(m)

    res = run_bass_kernel_spmd(nc, in_maps, core_ids=list(range(N_CORES)))
    out = np.concatenate([r["out"] for r in res.results], axis=0)
    return out.astype(np.float32)


# revision 23
# speedup vs baseline: 1.4307x; 1.2374x over previous
"""Trainium2 Bass kernel for nn_CategoryHead (tiny 4-layer post-norm
transformer classifier head over B=65536 samples, T=2 tokens, D=128).

Strategy: pure data-parallel over 8 NeuronCores (batch sharded 8192/core,
weights replicated). Feature-major activations ([128 feature partitions,
columns = sample-tokens]); the residual stream and all stream-path matmuls
stay fp32 (float32r full-rate mode), while attention internals (k-diff,
v-diff, tanh weights, o-premix) run bf16 where quantization error is
negligible.  Per-column LN stats are produced with ones-selector matmuls on
the PE (deferred two tiles behind the main per-tile chain so the PE queue
never head-of-line blocks on them), rsqrt via quake seed + 1 Newton step,
stats broadcast back with replicate matmuls.  Attention over T=2 uses
softmax-as-sigmoid: with xs=x0+x1, xd=x0-x1, the output is
o = 0.5*Wo^T Wv^T xs + Wo^T(tanh(d/2)_heads * 0.5*Wv^T xd); the xs-term
uses a host-fused weight (Wvo) accumulated straight into the output PSUM,
and the residual add rides the PE as an identity matmul into the same
accumulation group.  Squares for the variance stats run on the otherwise
idle GPSIMD engine; gelu is batched [128, 1024] on the Scalar engine.
"""

import numpy as np

L, T, D, H, NC_CLS = 4, 2, 128, 8, 7
DH = D // H
DFF = 4 * D
EPS = 1e-5
N_CORES = 8
B_FULL = 65536
B_CORE = B_FULL // N_CORES  # 8192
SAMP_PER_TILE = 256         # 256 samples -> 512 columns per tile
COLS = SAMP_PER_TILE * T    # 512

_CACHE = {}


def _build(b_core):
    import concourse.bacc as bacc
    import concourse.tile as tile
    import concourse.mybir as mybir
    from concourse import bass

    f32 = mybir.dt.float32
    f32r = mybir.dt.float32r
    bf16 = mybir.dt.bfloat16
    i32 = mybir.dt.int32
    AF = mybir.ActivationFunctionType
    OP = mybir.AluOpType

    n_tiles = b_core // SAMP_PER_TILE
    assert n_tiles <= 32

    nc = bacc.Bacc(
        "TRN2", target_bir_lowering=False, debug=False, num_devices=N_CORES
    )

    def din(name, shape, dt=f32):
        return nc.dram_tensor(name, shape, dt, kind="ExternalInput").ap()

    x_d = din("x", (D, 2, b_core), f32r)                 # feature-major chunks of x
    wproj_d = din("wproj", (D, 2, T, D), f32r)           # [fin_p, fin_chunk, tok, fout]
    wqkv_d = din("wqkv", (L, D, 3 * D), f32r)            # qkv_w[l].T
    wkvn_d = din("wkvn", (L, D, 2 * D), f32r)            # -qkv_w[l].T k/v slices
    wout_d = din("wout", (L, D, D), f32r)          # 0.5 * out_w[l].T
    wvo_d = din("wvo", (L, D, D), f32r)                  # 0.5 * (v_w out_w)[l].T fused
    wff1_d = din("wff1", (L, D, DFF), f32r)              # ff1_w[l].T
    wff2_d = din("wff2", (L, DFF, D), f32r)              # ff2_w[l].T
    wcls_d = din("wcls", (D, NC_CLS), bf16)        # cls_w.T
    btok_d = din("btok", (D, T))                   # pos_emb [d, t]
    zsel_d = din("zsel", (3, D, 2 * D), f32r)            # scatter sel 1/128,1/256,1/512
    rsel_d = din("rsel", (32, 32 * D), f32r)             # replicate selectors
    bhead_d = din("bhead", (D, H), bf16)           # 0.125 block-ones
    bbcast_d = din("bbcast", (H, D), bf16)         # head -> partitions block-ones
    ident_d = din("ident", (D, D), f32r)
    identf_d = din("identf", (32, 32))                 # identity for resid mms
    out_d = nc.dram_tensor("out", (b_core, NC_CLS), f32, kind="ExternalOutput").ap()

    def mm(out, lhsT, rhs, start=True, stop=True):
        nc.tensor.matmul(out, lhsT.bitcast(f32r), rhs.bitcast(f32r),
                         start=start, stop=stop)

    mmb = nc.tensor.matmul  # bf16 matmul (attention internals)

    def bcast_free(ap, n, axis=1):
        """Insert a stride-0 axis of size n at `axis` into a 2D AP."""
        return bass.AP(tensor=ap.tensor, offset=ap.offset,
                       ap=ap.ap[:axis] + [[0, n]] + ap.ap[axis:])

    with tile.TileContext(nc) as tc:
        with (
            tc.tile_pool(name="wpool", bufs=1) as wp,
            tc.tile_pool(name="resid", bufs=1) as rp,
            tc.tile_pool(name="stats", bufs=2) as stp,
            tc.tile_pool(name="stats1", bufs=2) as stp1,
            tc.tile_pool(name="work", bufs=3) as wk,
            tc.tile_pool(name="hpool", bufs=2) as hp,
            tc.tile_pool(name="sqpool", bufs=5) as sqp,
            tc.tile_pool(name="xin", bufs=3) as xp,
            tc.tile_pool(name="pstat", bufs=1, space="PSUM") as pstat,
            tc.tile_pool(name="pwork", bufs=6, space="PSUM") as pw,
        ):
            # ---- load weights/constants into SBUF (resident) ----
            wproj = wp.tile([D, 2, T, D], f32r)
            nc.sync.dma_start(out=wproj, in_=wproj_d)
            wqkv = wp.tile([D, L, 3 * D], f32r)
            nc.sync.dma_start(out=wqkv, in_=wqkv_d.rearrange("l p f -> p l f"))
            wkvn = wp.tile([D, L, 2 * D], f32r)
            nc.sync.dma_start(out=wkvn, in_=wkvn_d.rearrange("l p f -> p l f"))
            wout = wp.tile([D, L, D], f32r)
            nc.sync.dma_start(out=wout, in_=wout_d.rearrange("l p f -> p l f"))
            wvo = wp.tile([D, L, D], f32r)
            nc.sync.dma_start(out=wvo, in_=wvo_d.rearrange("l p f -> p l f"))
            wff1 = wp.tile([D, L, DFF], f32r)
            nc.sync.dma_start(out=wff1, in_=wff1_d.rearrange("l p f -> p l f"))
            wff2 = wp.tile([D, L, 4, D], f32r)
            nc.sync.dma_start(
                out=wff2, in_=wff2_d.rearrange("l (c p) f -> p l c f", p=D))
            wcls = wp.tile([D, NC_CLS], bf16)
            nc.sync.dma_start(out=wcls, in_=wcls_d)
            btok = wp.tile([D, T], f32)
            nc.sync.dma_start(out=btok, in_=btok_d)
            zsel = wp.tile([D, 3, 2 * D], f32r)
            nc.sync.dma_start(out=zsel, in_=zsel_d.rearrange("z p f -> p z f"))
            rsel = wp.tile([32, 32 * D], f32r)
            nc.sync.dma_start(out=rsel, in_=rsel_d)
            bhead = wp.tile([D, H], bf16)
            nc.sync.dma_start(out=bhead, in_=bhead_d)
            bbcast = wp.tile([H, D], bf16)
            nc.sync.dma_start(out=bbcast, in_=bbcast_d)
            ident = wp.tile([D, D], f32r)
            nc.sync.dma_start(out=ident, in_=ident_d)
            identf = wp.tile([32, 32], f32)
            nc.sync.dma_start(out=identf, in_=identf_d)
            magic = wp.tile([32, 1], i32)
            nc.vector.memset(magic, 0x5F3759DF)

            # residual stream, feature-major [d, tile, tok, samp], fp32
            tok = rp.tile([D, n_tiles, T, SAMP_PER_TILE], f32r)

            def zslice(z, i):
                return zsel[:, z, D - i: 2 * D - i]

            def rslice(i, gs):
                return rsel[:gs, i * D:(i + 1) * D]

            def ln_chain(s1, s2, nt, ncols=COLS):
                """Stats chain from PSUM mean rows s1[0:nt] and E[x^2] rows
                s2[0:nt].  Returns sbuf f32 (negmean, rstd) [32, ncols]."""
                negmean = stp.tile([32, COLS], f32r, tag="negmean")
                nc.scalar.activation(negmean[:nt, :ncols], s1[:nt, :ncols],
                                     AF.Identity, scale=-1.0)
                m2 = stp1.tile([32, COLS], f32, tag="m2")
                nc.vector.tensor_tensor(out=m2[:nt, :ncols],
                                        in0=negmean[:nt, :ncols],
                                        in1=negmean[:nt, :ncols], op=OP.mult)
                # u = (s2 + EPS) - m^2
                u = stp1.tile([32, COLS], f32, tag="u")
                nc.vector.scalar_tensor_tensor(
                    out=u[:nt, :ncols], in0=s2[:nt, :ncols], scalar=EPS,
                    in1=m2[:nt, :ncols], op0=OP.add, op1=OP.subtract)
                # quake rsqrt + 1 Newton iteration (fp32)
                y = stp.tile([32, COLS], i32, tag="y")
                nc.vector.tensor_scalar(out=y[:nt, :ncols],
                                        in0=u.bitcast(i32)[:nt, :ncols],
                                        scalar1=1, scalar2=None,
                                        op0=OP.logical_shift_right)
                nc.vector.tensor_tensor(
                    out=y[:nt, :ncols],
                    in0=bcast_free(magic[:nt, 0:1], ncols, axis=1),
                    in1=y[:nt, :ncols], op=OP.subtract)
                yf = y.bitcast(f32)
                t1 = stp1.tile([32, COLS], f32, tag="t1")
                nc.scalar.activation(t1[:nt, :ncols], yf[:nt, :ncols],
                                     AF.Square)
                nc.vector.tensor_tensor(out=t1[:nt, :ncols],
                                        in0=u[:nt, :ncols],
                                        in1=t1[:nt, :ncols], op=OP.mult)
                nc.vector.tensor_scalar(out=t1[:nt, :ncols],
                                        in0=t1[:nt, :ncols],
                                        scalar1=-0.5, scalar2=1.5,
                                        op0=OP.mult, op1=OP.add)
                rstd = stp.tile([32, COLS], f32r, tag="rstd")
                nc.vector.tensor_tensor(out=rstd[:nt, :ncols],
                                        in0=yf[:nt, :ncols],
                                        in1=t1[:nt, :ncols], op=OP.mult)
                return negmean, rstd

            def normalize(j, gs, dst, src_ap, negmean, rstd, ncols=COLS):
                """dst[...] = (src + repl(negmean_j)) * repl(rstd_j)"""
                mb = pw.tile([D, ncols], f32, tag="pwork")
                mm(mb, rslice(j, gs), negmean[:gs, :ncols])
                rb = pw.tile([D, ncols], f32, tag="pwork")
                mm(rb, rslice(j, gs), rstd[:gs, :ncols])
                cen = wk.tile([D, ncols], f32, tag="cen")
                nc.vector.tensor_tensor(out=cen, in0=src_ap, in1=mb, op=OP.add)
                nc.vector.tensor_tensor(out=dst, in0=cen, in1=rb, op=OP.mult)

            # Deferred stats: emit each tile's selector matmuls two tiles
            # late so the PE queue never blocks on the Act copy / GPSIMD
            # square that produce their inputs.
            class StatQ:
                def __init__(self, s1p, s2p, gs, z1=0, z2=0, ncols=COLS):
                    self.s1p, self.s2p, self.gs = s1p, s2p, gs
                    self.z1, self.z2, self.ncols = z1, z2, ncols
                    self.pend = []

                def push(self, j, src, sq):
                    self.pend.append((j, src, sq))
                    if len(self.pend) > 2:
                        self.emit_one()

                def emit_one(self):
                    j, src, sq = self.pend.pop(0)
                    g = self.gs
                    mm(self.s1p[:, :self.ncols], zslice(self.z1, j), src,
                       start=(j == 0), stop=(j == g - 1))
                    mm(self.s2p[:, :self.ncols], zslice(self.z2, j), sq,
                       start=(j == 0), stop=(j == g - 1))

                def flush(self):
                    while self.pend:
                        self.emit_one()

            # ============ phase 0: token projection ============
            for i in range(n_tiles):
                xt = xp.tile([D, 2, SAMP_PER_TILE], f32r, tag="xt")
                nc.sync.dma_start(
                    out=xt,
                    in_=x_d[:, :, i * SAMP_PER_TILE:(i + 1) * SAMP_PER_TILE])
                tk_ps = pw.tile([D, T, SAMP_PER_TILE], f32, tag="pwork")
                for t in range(T):
                    for fc in range(2):
                        mm(tk_ps[:, t, :], wproj[:, fc, t, :], xt[:, fc, :],
                           start=(fc == 0), stop=(fc == 1))
                for t in range(T):
                    nc.scalar.activation(tok[:, i, t, :], tk_ps[:, t, :],
                                         AF.Identity, bias=btok[:, t:t + 1])

            # ============ transformer layers ============
            GROUP = min(16, n_tiles)
            n_groups = (n_tiles + GROUP - 1) // GROUP
            groups = [list(range(g * GROUP, min((g + 1) * GROUP, n_tiles)))
                      for g in range(n_groups)]

            def frontA(lyr, j, i, gs, prev_stats, st):
                tki = tok[:, i, :, :]
                tkf = tki.rearrange("p t s -> p (t s)")
                if prev_stats is not None:
                    normalize(j, gs, tkf, tkf, *prev_stats)
                q_ps = pw.tile([D, T, SAMP_PER_TILE], f32, tag="pwork")
                qf_ps = q_ps.rearrange("p t s -> p (t s)")
                mm(qf_ps, wqkv[:, lyr, 0:D], tkf)
                # kd = Wk y0 - Wk y1, dv = Wv y0 - Wv y1 via +- accumulation
                # (shares the bank later reused for the o accumulation)
                ko_ps = pw.tile([D, 2, SAMP_PER_TILE], f32, tag="pwork")
                for c, (w0, w1) in enumerate((
                        (wqkv[:, lyr, D:2 * D], wkvn[:, lyr, 0:D]),
                        (wqkv[:, lyr, 2 * D:3 * D], wkvn[:, lyr, D:2 * D]))):
                    mm(ko_ps[:, c, :], w0, tki[:, 0, :],
                       start=True, stop=False)
                    mm(ko_ps[:, c, :], w1, tki[:, 1, :],
                       start=False, stop=True)
                kdv = wk.tile([D, 2, SAMP_PER_TILE], bf16, tag="kdv")
                nc.scalar.copy(kdv, ko_ps)
                st.update(tki=tki, tkf=tkf, kdv=kdv,
                          q_ps=q_ps, ko_ps=ko_ps)

            def midA(lyr, j, i, gs, st):
                q_ps = st["q_ps"]
                qf_ps = q_ps.rearrange("p t s -> p (t s)")
                qd = wk.tile([D, T, SAMP_PER_TILE], bf16, tag="qd")
                nc.vector.tensor_tensor(out=qd, in0=q_ps,
                                        in1=bcast_free(st["kdv"][:, 0, :], T),
                                        op=OP.mult)
                d_ps = qf_ps[:H, :]
                mmb(d_ps, bhead, qd.rearrange("p t s -> p (t s)"),
                    start=True, stop=True)
                th = wk.tile([H, COLS], bf16, tag="th")
                nc.scalar.activation(th, d_ps, AF.Tanh)
                st.update(th=th)

            def backA(lyr, j, i, gs, st, sk):
                tkf = st["tkf"]
                tb_ps = st["q_ps"]
                mmb(tb_ps.rearrange("p t s -> p (t s)"), bbcast, st["th"],
                    start=True, stop=True)
                opre = wk.tile([D, T, SAMP_PER_TILE], f32r, tag="opre")
                nc.vector.tensor_tensor(out=opre, in0=tb_ps,
                                        in1=bcast_free(st["kdv"][:, 1, :], T),
                                        op=OP.mult)
                o_ps = st["ko_ps"]
                of = o_ps.rearrange("p t s -> p (t s)")
                tki = st["tki"]
                for tt in range(T):
                    mm(of, wvo[:, lyr, :],
                       bcast_free(tki[:, tt, :], T),
                       start=(tt == 0), stop=False)
                mm(of, wout[:, lyr, :],
                   opre.rearrange("p t s -> p (t s)"),
                   start=False, stop=False)
                mm(of, ident, tkf, start=False, stop=True)
                nc.scalar.copy(tkf, of)
                sq = sqp.tile([D, COLS], f32r, tag="sq")
                nc.gpsimd.tensor_tensor(out=sq, in0=tkf, in1=tkf,
                                        op=OP.mult)
                sk.push(j, tkf, sq)

            def drive_rolling(gen_fns, width=2):
                live = []
                idx = 0
                while live or idx < len(gen_fns):
                    while len(live) < width and idx < len(gen_fns):
                        live.append(gen_fns[idx]())
                        idx += 1
                    for g in list(live):
                        try:
                            next(g)
                        except StopIteration:
                            live.remove(g)

            def emit_passA(lyr, tiles, prev_stats):
                gs = len(tiles)
                s1p = pstat.tile([D, COLS], f32, tag="s1")
                s2p = pstat.tile([D, COLS], f32, tag="s2")
                sk = StatQ(s1p, s2p, gs)
                sts = [dict() for _ in range(gs)]
                for j in range(gs + 2):
                    if j < gs:
                        frontA(lyr, j, tiles[j], gs, prev_stats, sts[j])
                    if 0 < j <= gs:
                        midA(lyr, j - 1, tiles[j - 1], gs, sts[j - 1])
                    if j > 1:
                        backA(lyr, j - 2, tiles[j - 2], gs, sts[j - 2], sk)
                sk.flush()
                return ln_chain(s1p, s2p, gs)

            def frontB(lyr, j, i, gs, stats1, st):
                tkf = tok[:, i, :, :].rearrange("p t s -> p (t s)")
                normalize(j, gs, tkf, tkf, *stats1)
                h = hp.tile([D, 4, COLS], f32r, tag="h_sb")
                for c in range(4):
                    h_ps = pw.tile([D, COLS], f32, tag="pwork")
                    mm(h_ps, wff1[:, lyr, c * D:(c + 1) * D], tkf)
                    nc.scalar.activation(h[:, c, :], h_ps, AF.Gelu)
                st.update(tkf=tkf, h=h)

            def backB(lyr, j, i, gs, st, sk):
                tkf, h = st["tkf"], st["h"]
                f_ps = pw.tile([D, COLS], f32, tag="pwork")
                for c in range(4):
                    mm(f_ps, wff2[:, lyr, c, :], h[:, c, :],
                       start=(c == 0), stop=False)
                mm(f_ps, ident, tkf, start=False, stop=True)
                nc.scalar.copy(tkf, f_ps)
                sq = sqp.tile([D, COLS], f32r, tag="sq")
                nc.gpsimd.tensor_tensor(out=sq, in0=tkf, in1=tkf,
                                        op=OP.mult)
                sk.push(j, tkf, sq)

            def emit_passB(lyr, tiles, stats1):
                gs = len(tiles)
                s1p = pstat.tile([D, COLS], f32, tag="s1")
                s2p = pstat.tile([D, COLS], f32, tag="s2")
                sk = StatQ(s1p, s2p, gs)
                sts = [dict() for _ in range(gs)]
                for j in range(gs + 1):
                    if j < gs:
                        frontB(lyr, j, tiles[j], gs, stats1, sts[j])
                    if j > 0:
                        backB(lyr, j - 1, tiles[j - 1], gs, sts[j - 1], sk)
                sk.flush()
                return ln_chain(s1p, s2p, gs)

            def gen_lnpass(j, i, gs, prev_stats, sk):
                tkf = tok[:, i, :, :].rearrange("p t s -> p (t s)")
                normalize(j, gs, tkf, tkf, *prev_stats)
                yield
                sq = sqp.tile([D, COLS], f32r, tag="sq")
                nc.gpsimd.tensor_tensor(out=sq, in0=tkf, in1=tkf,
                                        op=OP.mult)
                sk.push(j, tkf, sq)

            def emit_lnpass(tiles, prev_stats):
                gs = len(tiles)
                s1p = pstat.tile([D, COLS], f32, tag="s1")
                s2p = pstat.tile([D, COLS], f32, tag="s2")
                sk = StatQ(s1p, s2p, gs)
                gfs = [(lambda j=j: gen_lnpass(j, tiles[j], gs,
                                               prev_stats, sk))
                       for j in range(gs)]
                drive_rolling(gfs, width=3)
                sk.flush()
                return ln_chain(s1p, s2p, gs)

            def gen_H2(j, i, gs, statsf, sk):
                tki = tok[:, i, :, :]
                tkf = tki.rearrange("p t s -> p (t s)")
                normalize(j, gs, tkf, tkf, *statsf)
                yield
                nc.vector.tensor_tensor(out=tki[:, 0, :],
                                        in0=tki[:, 0, :],
                                        in1=tki[:, 1, :], op=OP.add)
                yield
                sq = sqp.tile([D, SAMP_PER_TILE], f32r, tag="sqh")
                nc.gpsimd.tensor_tensor(out=sq, in0=tki[:, 0, :],
                                        in1=tki[:, 0, :], op=OP.mult)
                sk.push(j, tki[:, 0, :], sq)

            def emit_H2(tiles, statsf):
                gs = len(tiles)
                s1p = pstat.tile([D, COLS], f32, tag="s1")
                s2p = pstat.tile([D, COLS], f32, tag="s2")
                sk = StatQ(s1p, s2p, gs, z1=1, z2=2, ncols=SAMP_PER_TILE)
                gfs = [(lambda j=j: gen_H2(j, tiles[j], gs, statsf, sk))
                       for j in range(gs)]
                drive_rolling(gfs, width=3)
                sk.flush()
                return ln_chain(s1p, s2p, gs, ncols=SAMP_PER_TILE)

            stats_p = [None] * n_groups
            for lyr in range(L):
                stats1 = [None] * n_groups
                for g in range(n_groups):
                    stats1[g] = emit_passA(lyr, groups[g], stats_p[g])
                for g in range(n_groups):
                    stats_p[g] = emit_passB(lyr, groups[g], stats1[g])

            # ============ head ============
            statsf = [None] * n_groups
            for g in range(n_groups):
                statsf[g] = emit_lnpass(groups[g], stats_p[g])
            statsc = [None] * n_groups
            for g in range(n_groups):
                statsc[g] = emit_H2(groups[g], statsf[g])

            # H3: cls_ln normalize + gelu + classifier + output
            def gen_H3(g, j, i):
                gs = len(groups[g])
                negmc, rstdc = statsc[g]
                p2 = tok[:, i, 0, :]
                mb = pw.tile([D, SAMP_PER_TILE], f32, tag="pwork")
                mm(mb, rslice(j, gs), negmc[:gs, :SAMP_PER_TILE])
                rb = pw.tile([D, SAMP_PER_TILE], f32, tag="pwork")
                mm(rb, rslice(j, gs), rstdc[:gs, :SAMP_PER_TILE])
                yield
                cen = wk.tile([D, SAMP_PER_TILE], f32, tag="cen")
                nc.vector.scalar_tensor_tensor(
                    out=cen, in0=p2, scalar=0.5, in1=mb,
                    op0=OP.mult, op1=OP.add)
                xh = wk.tile([D, SAMP_PER_TILE], bf16, tag="xh")
                nc.vector.tensor_tensor(out=xh, in0=cen, in1=rb, op=OP.mult)
                yield
                gl = wk.tile([D, SAMP_PER_TILE], bf16, tag="g")
                nc.scalar.activation(gl, xh, AF.Gelu)
                yield
                cls_ps = pw.tile([NC_CLS, SAMP_PER_TILE], f32, tag="pwork")
                mmb(cls_ps, wcls, gl, start=True, stop=True)
                yield
                cls_sb = wk.tile([NC_CLS, SAMP_PER_TILE], f32, tag="clssb")
                nc.scalar.copy(cls_sb, cls_ps)
                yield
                tr_ps = pw.tile([D, 2, NC_CLS], f32, tag="pwork")
                for sc in range(2):
                    nc.tensor.transpose(tr_ps[:, sc, :],
                                        cls_sb[:, sc * D:(sc + 1) * D],
                                        identf[:NC_CLS, :NC_CLS])
                yield
                obm = wk.tile([D, 2, NC_CLS], f32, tag="obm")
                nc.scalar.copy(obm, tr_ps)
                nc.sync.dma_start(
                    out=out_d[i * SAMP_PER_TILE:(i + 1) * SAMP_PER_TILE, :]
                    .rearrange("(sc p) c -> p sc c", p=D),
                    in_=obm)

            h3_gfs = []
            for g in range(n_groups):
                for j, i in enumerate(groups[g]):
                    h3_gfs.append(lambda g=g, j=j, i=i: gen_H3(g, j, i))
            drive_rolling(h3_gfs, width=2)

    nc.compile()
    return nc


def _to_bf16(a):
    import ml_dtypes
    return np.ascontiguousarray(np.asarray(a, np.float32)).astype(
        ml_dtypes.bfloat16)


def _f32(a):
    return np.ascontiguousarray(np.asarray(a, dtype=np.float32))


def _prep_weights(inputs):
    w = {}
    tp = np.asarray(inputs["token_proj_w"], np.float32).T  # [fin, fout]
    w["wproj"] = _f32(tp.reshape(2, D, T, D).transpose(1, 0, 2, 3))
    qkvT = np.asarray(inputs["qkv_w"], np.float32).transpose(0, 2, 1)
    w["wqkv"] = _f32(qkvT)
    w["wkvn"] = _f32(-qkvT[:, :, D:3 * D])
    outT = 0.5 * np.asarray(inputs["out_w"], np.float32).transpose(0, 2, 1)
    w["wout"] = _f32(outT)
    # Wvo[l] = (qkv_w[l].T)[:, 2D:3D] @ (0.5*out_w[l].T): x-space -> o-space
    w["wvo"] = _f32(np.einsum('lpv,lvo->lpo', qkvT[:, :, 2 * D:3 * D], outT))
    w["wff1"] = _f32(np.asarray(inputs["ff1_w"], np.float32).transpose(0, 2, 1))
    w["wff2"] = _f32(np.asarray(inputs["ff2_w"], np.float32).transpose(0, 2, 1))
    w["wcls"] = _to_bf16(np.asarray(inputs["cls_w"], np.float32).T)
    w["btok"] = _f32(
        np.asarray(inputs["pos_emb"], np.float32)[0].T
        + np.asarray(inputs["token_proj_b"], np.float32).reshape(T, D).T)
    zsel = np.zeros((3, D, 2 * D), dtype=np.float32)
    zsel[0, :, D] = 1.0 / 128
    zsel[1, :, D] = 1.0 / 256
    zsel[2, :, D] = 1.0 / 512
    w["zsel"] = zsel
    rsel = np.zeros((32, 32 * D), dtype=np.float32)
    for i in range(32):
        rsel[i, i * D:(i + 1) * D] = 1.0
    w["rsel"] = rsel
    bhead = np.zeros((D, H), dtype=np.float32)
    for h in range(H):
        bhead[h * DH:(h + 1) * DH, h] = 0.125
    w["bhead"] = _to_bf16(bhead)
    w["bbcast"] = _to_bf16((bhead.T != 0).astype(np.float32))
    w["ident"] = np.eye(D, dtype=np.float32)
    w["identf"] = np.eye(32, dtype=np.float32)

    # Unused-by-construction inputs (all zeros / ones in this model family);
    # verify that so silently ignoring them is sound.
    for name in ("qkv_b", "out_b", "ff1_b", "ff2_b", "cls_b"):
        assert not np.any(inputs[name]), f"{name} expected to be all zeros"
    for name in ("ln1_w", "ln2_w", "lnf_w", "cls_ln_w"):
        assert np.all(np.asarray(inputs[name]) == 1.0), \
            f"{name} expected to be all ones"
    for name in ("ln1_b", "ln2_b", "lnf_b", "cls_ln_b"):
        assert not np.any(inputs[name]), f"{name} expected to be all zeros"
    return w


def kernel(**inputs):
    from concourse.bass_utils import run_bass_kernel_spmd

    x = np.asarray(inputs["x"], dtype=np.float32).reshape(B_FULL, T * D)
    if "nc" not in _CACHE:
        _CACHE["nc"] = _build(B_CORE)
    nc = _CACHE["nc"]

    w = _prep_weights(inputs)
    in_maps = []
    for c in range(N_CORES):
        xc = x[c * B_CORE:(c + 1) * B_CORE]            # [b, 256]
        xf = xc.reshape(B_CORE, 2, D).transpose(2, 1, 0)  # [D, 2, b]
        m = dict(w)
        m["x"] = _f32(xf)
        in_maps.append(m)

    res = run_bass_kernel_spmd(nc, in_maps, core_ids=list(range(N_CORES)))
    out = np.concatenate([r["out"] for r in res.results], axis=0)
    return out.astype(np.float32)
